# revision 17
# baseline (speedup 1.0000x reference)
"""Trainium2 Bass kernel for nn_AdvancedRNN.

Reference semantics (H=512, B=1024, T=256 warmup, S=64 rollout):
  Phase 1: h = relu(x_t * w_in + h @ W_hh.T + bias)   for t in 0..T-1, h0 = 0
  Phase 2: y = h @ W_fc.T + b_fc ; h = relu(y @ W_ih.T + h @ W_hh.T + bias)
           output ys[:, s, 0] = y   for s in 0..S-1

Strategy: data-parallel over batch across 8 cores (128 batch rows each),
weights replicated.  On each core the hidden state is kept TRANSPOSED in
SBUF as hT[hidden, batch] = 4 chunks of [128, 128] fp16, so the per-step
matmul  hT_new[i,b] = sum_j W_hh[i,j] * hT[j,b]  uses W_hh.T tiles as the
stationary operand and hT chunks as the moving operand, producing the next
state already in the layout the following step consumes (no transposes in
the loop).  PSUM accumulates in fp32.

The warmup-phase rank-1 injection w_in[i]*x_t[b] is precomputed on the host
for all t, streamed in by DMA as fp16 windows, and added to the PSUM result
on the vector engine; the ReLU+bias+fp16-downcast back to SBUF is split
between the scalar and vector engines so neither saturates.  Each PSUM
bank's accumulation group is ordered to END on the oldest-produced h chunk,
which lets consecutive time steps overlap on the tensor engine.  The
rollout-phase y feedback stays on the PE as K=1 rank-1 matmuls (y is only
known at runtime).
"""

import numpy as np

H = 512          # hidden
B = 1024         # global batch
T = 256          # warmup steps
S = 64           # rollout steps
NCORES = 8
BL = B // NCORES # local batch = 128
HC = H // 128    # hidden chunks = 4


def _build(nc, bfc_val, T_steps=T, S_steps=S, x_on_dve=False, y_on_dve=False,
           repeats=1, hw_loop=False, x_dma=True):
    import concourse.mybir as mybir
    from concourse.bass import ts
    from concourse.tile import TileContext

    fp16 = mybir.dt.float16
    fp32 = mybir.dt.float32
    RELU = mybir.ActivationFunctionType.Relu
    ADD = mybir.AluOpType.add
    MAX = mybir.AluOpType.max

    def relu_out(hn, src, ic):
        # chunks 0,1 on ACT; 2,3 on DVE (tensor_scalar add-bias + max0) so
        # neither engine saturates (DVE also carries the x-injection adds)
        if ic < 2:
            nc.scalar.activation(hn[:], src[:], RELU, bias=bias[:, ic:ic + 1])
        else:
            nc.vector.tensor_scalar(hn[:], src[:], bias[:, ic:ic + 1], 0.0,
                                    op0=ADD, op1=MAX)

    # ---- DRAM I/O (host pre-packs layouts; see kernel() below) ----
    xT_d   = nc.declare_dram_parameter("xT",   [1, T_steps * BL], fp16, isOutput=False)
    whh_d  = nc.declare_dram_parameter("whh",  [128, HC * HC * 128], fp16, isOutput=False)
    win_d  = nc.declare_dram_parameter("win",  [1, H], fp16, isOutput=False)
    winc_d = nc.declare_dram_parameter("winc", [128, HC], fp32, isOutput=False)
    wfc_d  = nc.declare_dram_parameter("wfc",  [128, HC], fp16, isOutput=False)
    bias_d = nc.declare_dram_parameter("bias", [128, HC], fp32, isOutput=False)
    ys_d   = nc.declare_dram_parameter("ys",   [1, S_steps * BL], fp32, isOutput=True)
    if x_dma:
        xc_d = nc.declare_dram_parameter("xc", [128, T_steps * HC * BL], fp16,
                                         isOutput=False)

    with TileContext(nc) as tc:
        with (
            tc.tile_pool(name="const", bufs=1) as const,
            tc.tile_pool(name="hpool", bufs=12) as hpool,
            tc.tile_pool(name="vpool", bufs=16) as vpool,
            tc.tile_pool(name="ppool", bufs=8, space="PSUM") as ppool,
        ):
            # ---- load constants ----
            xT = const.tile([1, T_steps * BL], fp16)
            nc.sync.dma_start(out=xT[:], in_=xT_d[:])
            whh = const.tile([128, HC * HC * 128], fp16)
            nc.sync.dma_start(out=whh[:], in_=whh_d[:])
            win = const.tile([1, H], fp16)
            nc.sync.dma_start(out=win[:], in_=win_d[:])
            winc = const.tile([128, HC], fp32)
            nc.sync.dma_start(out=winc[:], in_=winc_d[:])
            wfc = const.tile([128, HC], fp16)
            nc.sync.dma_start(out=wfc[:], in_=wfc_d[:])
            bias = const.tile([128, HC], fp32)
            nc.sync.dma_start(out=bias[:], in_=bias_d[:])
            ystrip = const.tile([1, S_steps * BL], fp32)

            # ---- prime engine clocks against the const DMAs so steady-state
            # instructions need at most one sync wait (ISA limit) ----
            scr_a = const.tile([128, 1], fp32)
            nc.scalar.copy(out=scr_a[:], in_=bias[:, 0:1])
            scr_v = const.tile([128, 1], fp32)
            nc.vector.tensor_copy(scr_v[:], winc[:, 0:1])
            scr_p = ppool.tile([128, 1], fp32, tag="ps")
            nc.tensor.matmul(scr_p[:], whh[:, 0:128], whh[:, 0:1],
                             start=True, stop=True)
            nc.tensor.matmul(scr_p[0:1, 0:1], win[0:1, 0:1], xT[0:1, 0:1],
                             start=True, stop=True)
            nc.tensor.matmul(scr_p[0:1, 0:1], wfc[:, 0:1], whh[:, 0:1],
                             start=True, stop=True)

            import contextlib
            rep_ctx = (tc.For_i(0, repeats, 1) if hw_loop
                       else contextlib.nullcontext(0))
            with rep_ctx as _rep_iv:
              for _rep in range(1 if hw_loop else repeats):
                h_prev = None  # h0 == 0: step 0 skips the W_hh matmuls

                def inject_dve(row_ap, pss, h_new):
                    """rank-1 w_in[i]*row[b] via gpsimd bcast + DVE, then relu."""
                    vb = vpool.tile([128, BL], fp16, tag="vb")
                    nc.gpsimd.partition_broadcast(vb[:], row_ap)
                    for ic in range(HC):
                        vc = vpool.tile([128, BL], fp16, tag="vc")
                        nc.vector.tensor_scalar_mul(vc[:], vb[:], winc[:, ic:ic + 1])
                        pre = vpool.tile([128, BL], fp16, tag="pre")
                        nc.vector.tensor_tensor(pre[:], pss[ic][:], vc[:], op=ADD)
                        hn = hpool.tile([128, BL], fp16, tag="h")
                        nc.scalar.activation(hn[:], pre[:], RELU,
                                             bias=bias[:, ic:ic + 1])
                        h_new.append(hn)

                # ---- phase 1: warmup over x ----
                XCW = 16  # steps per xc DMA window
                xc_tiles = {}
                for t in range(T_steps):
                    xrow = xT[0:1, ts(t, BL)]
                    h_new = []
                    if x_dma:
                        if t % XCW == 0:
                            w = min(XCW, T_steps - t)
                            xcw = vpool.tile([128, w * HC * BL], fp16, tag="xcw",
                                             bufs=3)
                            nc.sync.dma_start(
                                out=xcw[:],
                                in_=xc_d[:, t * HC * BL:(t + w) * HC * BL])
                            xc_tiles = {t + i: xcw[:, ts(i, HC * BL)]
                                        for i in range(w)}
                        xc_t = xc_tiles[t]
                        for ic in range(HC):
                            hn = hpool.tile([128, BL], fp16, tag="h")
                            if t == 0:
                                relu_out(hn, xc_t[:, ts(ic, BL)], ic)
                                h_new.append(hn)
                                continue
                            ps = ppool.tile([128, BL], fp32, tag="ps")
                            korder = [(ic + 1 + k) % HC for k in range(HC)]
                            for n, jc in enumerate(korder):
                                nc.tensor.matmul(ps[:], whh[:, ts(ic * HC + jc, 128)],
                                                 h_prev[jc][:],
                                                 start=(n == 0), stop=(n == HC - 1))
                            pre = vpool.tile([128, BL], fp16, tag="pre")
                            nc.vector.tensor_tensor(pre[:], ps[:],
                                                    xc_t[:, ts(ic, BL)], op=ADD)
                            relu_out(hn, pre, ic)
                            h_new.append(hn)
                        h_prev = h_new
                        continue
                    if x_on_dve:
                        pss = []
                        for ic in range(HC):
                            ps = ppool.tile([128, BL], fp32, tag="ps")
                            for jc in range(HC):
                                nc.tensor.matmul(ps[:], whh[:, ts(ic * HC + jc, 128)],
                                                 h_prev[jc][:],
                                                 start=(jc == 0), stop=(jc == HC - 1))
                            pss.append(ps)
                        inject_dve(xrow, pss, h_new)
                    else:
                        for ic in range(HC):
                            ps = ppool.tile([128, BL], fp32, tag="ps")
                            nc.tensor.matmul(ps[:], win[0:1, ts(ic, 128)], xrow,
                                             start=True, stop=(t == 0))
                            # end each bank's group on the OLDEST h chunk so
                            # consecutive steps overlap on the PE
                            if t > 0:
                                korder = [(ic + 1 + k) % HC for k in range(HC)]
                                for n, jc in enumerate(korder):
                                    nc.tensor.matmul(
                                        ps[:], whh[:, ts(ic * HC + jc, 128)],
                                        h_prev[jc][:],
                                        start=False, stop=(n == HC - 1))
                            hn = hpool.tile([128, BL], fp16, tag="h")
                            relu_out(hn, ps, ic)
                            h_new.append(hn)
                    h_prev = h_new

                # ---- phase 2: autoregressive rollout ----
                for s in range(S_steps):
                    h_new = []
                    if y_on_dve:
                        pss = []
                        for ic in range(HC):
                            ps = ppool.tile([128, BL], fp32, tag="ps")
                            for jc in range(HC):
                                nc.tensor.matmul(ps[:], whh[:, ts(ic * HC + jc, 128)],
                                                 h_prev[jc][:],
                                                 start=(jc == 0), stop=(jc == HC - 1))
                            pss.append(ps)
                        inject_dve(y16[0:1, :], pss, h_new)
                    else:
                        pss = []
                        psy = None
                        y16 = None
                        for ic in range(HC):
                            ps = ppool.tile([128, BL], fp32, tag="ps")
                            korder = [(ic + 1 + k) % HC for k in range(HC)]
                            for n, jc in enumerate(korder):
                                nc.tensor.matmul(ps[:], whh[:, ts(ic * HC + jc, 128)],
                                                 h_prev[jc][:],
                                                 start=(n == 0), stop=False)
                            pss.append(ps)
                            if ic == 1:
                                # fc group: y = W_fc @ h + b_fc (PSUM [1, BL]),
                                # emitted mid-step so its chunk-3 read and the
                                # DVE copy are off the PE critical path
                                psy = ppool.tile([1, BL], fp32, tag="ps")
                                for kc in range(HC):
                                    nc.tensor.matmul(psy[:], wfc[:, kc:kc + 1],
                                                     h_prev[kc][:],
                                                     start=(kc == 0),
                                                     stop=(kc == HC - 1))
                                y16 = vpool.tile([1, BL], fp16, tag="y16")
                                nc.vector.tensor_scalar_add(y16[:], psy[:],
                                                            float(bfc_val))
                                nc.vector.tensor_scalar_add(
                                    ystrip[0:1, ts(s, BL)], psy[:], float(bfc_val))
                        for ic in range(HC):
                            nc.tensor.matmul(pss[ic][:], win[0:1, ts(ic, 128)],
                                             y16[:], start=False, stop=True)
                            hn = hpool.tile([128, BL], fp16, tag="h")
                            relu_out(hn, pss[ic], ic)
                            h_new.append(hn)
                    h_prev = h_new

            nc.sync.dma_start(out=ys_d[:], in_=ystrip[:])
    return nc


def _pack_inputs(x, W_ih, W_hh, b_ih, b_hh, W_fc, b_fc, want_xc=False):
    """Host-side layout prep. Returns (shared, per_core_xT, bfc_val[, xcs])."""
    x = np.asarray(x, np.float32)
    W_ih = np.asarray(W_ih, np.float32)
    W_hh = np.asarray(W_hh, np.float32)
    W_fc = np.asarray(W_fc, np.float32)
    b = (np.asarray(b_ih, np.float32) + np.asarray(b_hh, np.float32))

    WT = W_hh.T  # WT[j, i] = W_hh[i, j]
    whh = np.zeros([128, HC * HC * 128], np.float16)
    for ic in range(HC):
        for jc in range(HC):
            whh[:, (ic * HC + jc) * 128:(ic * HC + jc + 1) * 128] = \
                WT[jc * 128:(jc + 1) * 128, ic * 128:(ic + 1) * 128]
    win = W_ih[:, 0][None, :].astype(np.float16)            # [1, H]
    winc = W_ih[:, 0].reshape(HC, 128).T.astype(np.float32) # [128, HC]
    wfc = W_fc[0].reshape(HC, 128).T.astype(np.float16)     # [128, HC]
    bias = b.reshape(HC, 128).T.astype(np.float32)          # [128, HC]
    bfc_val = float(np.asarray(b_fc, np.float32).reshape(-1)[0])

    shared = {"whh": whh, "win": win, "winc": winc, "wfc": wfc, "bias": bias}
    xTs = []
    xcs = []
    wr = W_ih[:, 0].reshape(HC, 128)
    for c in range(NCORES):
        xl = x[c * BL:(c + 1) * BL, :]                      # [BL, T]
        xlT = np.ascontiguousarray(xl.T)                    # [T, BL]
        xT = xlT.astype(np.float16).reshape(1, -1)
        xTs.append(xT)
        if want_xc:
            A = wr[None, :, :, None] * xlT[:, None, None, :]   # [T, HC, 128, BL]
            xc = np.ascontiguousarray(
                A.transpose(2, 0, 1, 3).reshape(128, -1)).astype(np.float16)
            xcs.append(xc)
    if want_xc:
        return shared, xTs, bfc_val, xcs
    return shared, xTs, bfc_val


def _make_nc(bfc_val, **kw):
    from concourse import bacc
    nc = bacc.Bacc()
    _build(nc, bfc_val, **kw)
    nc.compile()
    return nc


def kernel(x, W_ih, W_hh, b_ih, b_hh, W_fc, b_fc, num_steps):
    from concourse.bass_utils import run_bass_kernel_spmd

    assert int(num_steps) == S, f"kernel hardcodes num_steps={S}"
    shared, xTs, bfc_val, xcs = _pack_inputs(x, W_ih, W_hh, b_ih, b_hh,
                                             W_fc, b_fc, want_xc=True)

    nc = _make_nc(bfc_val)

    in_maps = [dict(shared, xT=xTs[c], xc=xcs[c]) for c in range(NCORES)]
    res = run_bass_kernel_spmd(nc, in_maps, list(range(NCORES)))

    # ys strip per core: [1, S*BL] with layout [s, b] -> [BL, S]
    outs = []
    for c in range(NCORES):
        ys = np.asarray(res.results[c]["ys"], np.float32).reshape(S, BL)
        outs.append(ys.T)                                   # [BL, S]
    out = np.concatenate(outs, axis=0)                      # [B, S]
    return out[:, :, None].astype(np.float32)               # [B, S, 1]

